# revision 17
# baseline (speedup 1.0000x reference)
"""Trainium2 Bass kernel for nn_FMREDynamicDropout.

reference semantics:
    scaled = (fi - fi.min()) / (fi.max() - fi.min())
    rates  = 0.1 + 0.4 * (1 - scaled)              # [C]
    keep_p = 1 - rates                             # [C]
    mask   = jax.random.bernoulli(key(42), keep_p[None,:,None,None], x.shape)
    out    = x * mask

jax.random.bernoulli(key, p, shape) == jax.random.uniform(key, shape, f32) < p,
and the uniform table u depends only on the (constant) key and shape, so it
is precomputed once on host.  u = m / 2^23 with integer m < 2^23, and
u < p  <=>  m < p*2^23 =: tau.  Writing uhat = m >> 15 (8 bits) and
tauhat_c = floor(p_c * 2^8):
    uhat < tauhat_c  -> mask = 1      (certain)
    uhat > tauhat_c  -> mask = 0      (certain)
    uhat == tauhat_c -> undecided     (prob 2^-8: ~150K of 38.5M elements)
The device streams only uhat (1B instead of 4B per element) and computes
out = (uhat < tauhat) * x in ONE fused DVE op; the ~0.4% undecided elements
are patched exactly on host via a precomputed per-channel counting-sort
index of uhat.  The result stays bit-identical to the reference.

Device layout: each row (one (b, c) HW-plane of 3136 px) is packed
host-side as bytes [x: 3136 f32 | uhat: 3136 u8 | tauhat: 1 f32] = 15684 B.
One DMA per 128-row tile brings everything the fused op needs, so every
instruction carries at most ONE sync wait (this walrus build rejects
instructions with more than one).  Work is sharded batch-wise: 4 of 32
batches per NeuronCore; raw-Bass 3-stage pipeline (SP HWDGE loads -> DVE
fused op -> ACT HWDGE stores) with per-slot semaphores (cumulative
single-sem thresholds are racy under out-of-order DMA completion).
"""

import os
import sys

import numpy as np

for _p in ("/opt/trn_rl_repo",):
    if _p not in sys.path and os.path.isdir(_p):
        sys.path.insert(0, _p)

_B, _C, _H, _W = 32, 384, 56, 56
_NCORES = 8
_BPC = _B // _NCORES          # batches per core: 4
_ROWS = _BPC * _C             # 1536 rows per core, row = (b, c) plane
_COLS = _H * _W               # 3136
_PT = _ROWS // 128            # 12 partition tiles per core
_XB = 4 * _COLS               # 12544: x bytes per row
_UB = _COLS                   # 3136:  uhat bytes per row
_PACKB = _XB + _UB + 4        # 15684: x | uhat | tauhat
_NPC = _B * _COLS             # 100352 elements per channel

TRACE = False                 # test harness may flip this for profiling
_state = {}


def _get_tables():
    """One-time constants: uniform table, uhat, per-channel sorted index."""
    if "u" in _state:
        return _state
    import jax
    import jax.numpy as jnp

    u = np.asarray(
        jax.random.uniform(jax.random.key(42), (_B, _C, _H, _W), dtype=jnp.float32)
    )
    # u = m / 2^23 exactly; recover m and its top 16 bits
    m = (u * np.float32(8388608.0)).astype(np.uint32)      # exact
    uhat = (m >> 15).astype(np.uint8)
    _state["u"] = u
    _state["uhat"] = uhat.reshape(_NCORES, _ROWS, _COLS)

    # per-channel sorted-by-uhat index for exact boundary patching
    uc = np.ascontiguousarray(u.transpose(1, 0, 2, 3)).reshape(_C, _NPC)
    uhc = np.ascontiguousarray(
        uhat.reshape(_B, _C, _COLS).transpose(1, 0, 2)
    ).reshape(_C, _NPC)
    order = np.argsort(uhc, axis=1, kind="stable")
    _state["sorted_u8"] = np.take_along_axis(uhc, order, 1)
    _state["sorted_u"] = np.take_along_axis(uc, order, 1)
    # global flat index of each sorted element: pos = b*COLS + hw within
    # channel c -> global = b*(C*COLS) + c*COLS + hw
    _state["g_idx"] = (
        (order // _COLS) * (_C * _COLS)
        + np.arange(_C, dtype=np.int64)[:, None] * _COLS
        + (order % _COLS)
    ).astype(np.int64)
    return _state


def _keep_prob(fi):
    """Replicates the reference's f32 arithmetic for keep_prob exactly."""
    import jax.numpy as jnp

    fij = jnp.asarray(fi, dtype=jnp.float32)
    scaled = (fij - fij.min()) / (fij.max() - fij.min())
    rates = 0.1 + (0.5 - 0.1) * (1.0 - scaled)
    return np.asarray(1.0 - rates, dtype=np.float32)


def _get_buf():
    """Persistent packed input bytes [cores, rows, x|uhat|tauhat]."""
    if "buf" not in _state:
        st = _get_tables()
        buf = np.empty((_NCORES, _ROWS, _PACKB), dtype=np.uint8)
        buf[:, :, _XB : _XB + _UB] = st["uhat"]
        _state["buf"] = buf
    return _state["buf"]


def _build_nc():
    from contextlib import ExitStack

    import concourse.bass as bass
    import concourse.mybir as mybir

    nc = bass.Bass()
    f32 = mybir.dt.float32
    xu = nc.declare_dram_parameter("xu", [_ROWS, _PACKB], mybir.dt.uint8,
                                   isOutput=False)
    out = nc.declare_dram_parameter("out", [_ROWS, _COLS], f32, isOutput=True)
    B = 6  # pipeline depth (SBUF slots)

    with (
        nc.sbuf_tensor([128, B * _PACKB], mybir.dt.uint8) as tiles,
        nc.semaphore("dve_sem") as dve_sem,
        ExitStack() as ctx,
        nc.Block() as block,
    ):
        load_sems = [
            ctx.enter_context(nc.semaphore(f"load{s}")) for s in range(B)
        ]
        store_sems = [
            ctx.enter_context(nc.semaphore(f"store{s}")) for s in range(B)
        ]

        def slot(t):
            base = (t % B) * _PACKB
            return (
                tiles[:, base : base + _XB].bitcast(f32),            # x
                tiles[:, base + _XB : base + _XB + _UB],             # uhat u8
                tiles[:, base + _XB + _UB : base + _PACKB].bitcast(f32),
                tiles[:, base : base + _PACKB],                      # raw slot
            )

        @block.sync
        def _(g):
            for t in range(_PT):
                s = t % B
                if t >= B:
                    # slot free once its previous store finished reading
                    g.wait_ge(store_sems[s], 16 * (t // B))
                g.dma_start(slot(t)[3], xu[bass.ts(t, 128), :]).then_inc(
                    load_sems[s], 16
                )

        @block.vector
        def _(v):
            for t in range(_PT):
                s = t % B
                v.wait_ge(load_sems[s], 16 * (t // B + 1))
                xs, us, ks, _raw = slot(t)
                v.scalar_tensor_tensor(
                    out=xs,
                    in0=us,
                    scalar=ks,
                    in1=xs,
                    op0=mybir.AluOpType.is_lt,
                    op1=mybir.AluOpType.mult,
                ).then_inc(dve_sem, 1)

        @block.scalar
        def _(s_eng):
            for t in range(_PT):
                s_eng.wait_ge(dve_sem, t + 1)
                s_eng.dma_start(out[bass.ts(t, 128), :], slot(t)[0]).then_inc(
                    store_sems[t % B], 16
                )
    return nc


def kernel(x, feature_importance):
    from concourse.bass_utils import run_bass_kernel_spmd

    if "nc" not in _state:
        _state["nc"] = _build_nc()
    nc = _state["nc"]

    st = _get_tables()
    buf = _get_buf()
    kp = _keep_prob(np.asarray(feature_importance))
    tauhat64 = np.floor(kp.astype(np.float64) * 256.0)     # exact
    tauhat32 = tauhat64.astype(np.float32)                 # <= 230, exact
    tauhat8 = tauhat64.astype(np.uint8)

    x_np = np.asarray(x, dtype=np.float32)
    f32v = buf.view(np.float32).reshape(_NCORES, _ROWS, _PACKB // 4)
    f32v[:, :, :_COLS] = x_np.reshape(_NCORES, _ROWS, _COLS)
    f32v[:, :, _PACKB // 4 - 1] = np.tile(tauhat32, _BPC)[None, :]

    in_maps = [{"xu": buf[k]} for k in range(_NCORES)]
    res = run_bass_kernel_spmd(
        nc, in_maps, core_ids=list(range(_NCORES)), trace=TRACE
    )
    _state["last"] = res
    out = np.concatenate([res.results[k]["out"] for k in range(_NCORES)], axis=0)
    outf = out.reshape(-1)

    # exact host patch of the undecided boundary bucket (uhat == tauhat_c)
    xf = x_np.reshape(-1)
    s8, sv, gi = st["sorted_u8"], st["sorted_u"], st["g_idx"]
    for c in range(_C):
        lo = np.searchsorted(s8[c], tauhat8[c], "left")
        hi = np.searchsorted(s8[c], tauhat8[c], "right")
        if lo == hi:
            continue
        g = gi[c, lo:hi]
        mask = (sv[c, lo:hi] < kp[c]).astype(np.float32)
        outf[g] = xf[g] * mask

    return out.reshape(_B, _C, _H, _W)


# revision 18
# speedup vs baseline: 1.1520x; 1.1520x over previous
"""Trainium2 Bass kernel for nn_FMREDynamicDropout.

reference semantics:
    scaled = (fi - fi.min()) / (fi.max() - fi.min())
    rates  = 0.1 + 0.4 * (1 - scaled)              # [C]
    keep_p = 1 - rates                             # [C]
    mask   = jax.random.bernoulli(key(42), keep_p[None,:,None,None], x.shape)
    out    = x * mask

jax.random.bernoulli(key, p, shape) == jax.random.uniform(key, shape, f32) < p,
and the uniform table u depends only on the (constant) key and shape, so it
is precomputed once on host.  u = m / 2^23 with integer m < 2^23, and
u < p  <=>  m < p*2^23 =: tau.  Writing uhat = m >> 15 (8 bits) and
tauhat_c = floor(p_c * 2^8):
    uhat < tauhat_c  -> mask = 1      (certain)
    uhat > tauhat_c  -> mask = 0      (certain)
    uhat == tauhat_c -> undecided     (prob 2^-8: ~150K of 38.5M elements)
The device streams only uhat (1B instead of 4B per element) and computes
out = (uhat < tauhat) * x in ONE fused DVE op; the ~0.4% undecided elements
are patched exactly on host via a precomputed per-channel counting-sort
index of uhat.  The result stays bit-identical to the reference.

Device layout: each row (one (b, c) HW-plane of 3136 px) is packed
host-side as bytes [x: 3136 f32 | uhat: 3136 u8 | tauhat: 1 f32] = 15684 B.
One DMA per 128-row tile brings everything the fused op needs, so every
instruction carries at most ONE sync wait (this walrus build rejects
instructions with more than one).  Work is sharded batch-wise: 4 of 32
batches per NeuronCore; raw-Bass 3-stage pipeline (SP HWDGE loads -> DVE
fused op -> ACT HWDGE stores) with per-slot semaphores (cumulative
single-sem thresholds are racy under out-of-order DMA completion).
"""

import os
import sys

import numpy as np

for _p in ("/opt/trn_rl_repo",):
    if _p not in sys.path and os.path.isdir(_p):
        sys.path.insert(0, _p)

_B, _C, _H, _W = 32, 384, 56, 56
_NCORES = 8
_BPC = _B // _NCORES          # batches per core: 4
_ROWS = _BPC * _C             # 1536 rows per core, row = (b, c) plane
_COLS = _H * _W               # 3136
_PT = _ROWS // 128            # 12 partition tiles per core
_XB = 4 * _COLS               # 12544: x bytes per row
_UB = _COLS                   # 3136:  uhat bytes per row
_PACKB = _XB + _UB + 4        # 15684: x | uhat | tauhat
_NPC = _B * _COLS             # 100352 elements per channel

TRACE = False                 # test harness may flip this for profiling
_state = {}


def _get_tables():
    """One-time constants: uniform table, uhat, per-channel sorted index."""
    if "u" in _state:
        return _state
    import jax
    import jax.numpy as jnp

    u = np.asarray(
        jax.random.uniform(jax.random.key(42), (_B, _C, _H, _W), dtype=jnp.float32)
    )
    # u = m / 2^23 exactly; recover m and its top 16 bits
    m = (u * np.float32(8388608.0)).astype(np.uint32)      # exact
    uhat = (m >> 15).astype(np.uint8)
    _state["u"] = u
    _state["uhat"] = uhat.reshape(_NCORES, _ROWS, _COLS)

    # per-channel sorted-by-uhat index for exact boundary patching
    uc = np.ascontiguousarray(u.transpose(1, 0, 2, 3)).reshape(_C, _NPC)
    uhc = np.ascontiguousarray(
        uhat.reshape(_B, _C, _COLS).transpose(1, 0, 2)
    ).reshape(_C, _NPC)
    order = np.argsort(uhc, axis=1, kind="stable")
    _state["sorted_u8"] = np.take_along_axis(uhc, order, 1)
    _state["sorted_u"] = np.take_along_axis(uc, order, 1)
    # global flat index of each sorted element: pos = b*COLS + hw within
    # channel c -> global = b*(C*COLS) + c*COLS + hw
    _state["g_idx"] = (
        (order // _COLS) * (_C * _COLS)
        + np.arange(_C, dtype=np.int64)[:, None] * _COLS
        + (order % _COLS)
    ).astype(np.int64)
    return _state


def _keep_prob(fi):
    """Replicates the reference's f32 arithmetic for keep_prob exactly."""
    import jax.numpy as jnp

    fij = jnp.asarray(fi, dtype=jnp.float32)
    scaled = (fij - fij.min()) / (fij.max() - fij.min())
    rates = 0.1 + (0.5 - 0.1) * (1.0 - scaled)
    return np.asarray(1.0 - rates, dtype=np.float32)


def _get_buf():
    """Persistent packed input bytes [cores, rows, x|uhat|tauhat]."""
    if "buf" not in _state:
        st = _get_tables()
        buf = np.empty((_NCORES, _ROWS, _PACKB), dtype=np.uint8)
        buf[:, :, _XB : _XB + _UB] = st["uhat"]
        _state["buf"] = buf
    return _state["buf"]


def _build_nc():
    from contextlib import ExitStack

    import concourse.bass as bass
    import concourse.mybir as mybir

    nc = bass.Bass()
    f32 = mybir.dt.float32
    xu = nc.declare_dram_parameter("xu", [_ROWS, _PACKB], mybir.dt.uint8,
                                   isOutput=False)
    out = nc.declare_dram_parameter("out", [_ROWS, _COLS], f32, isOutput=True)
    B = 8  # pipeline depth (SBUF slots)

    with (
        nc.sbuf_tensor([128, B * _PACKB], mybir.dt.uint8) as tiles,
        nc.semaphore("dve_sem") as dve_sem,
        ExitStack() as ctx,
        nc.Block() as block,
    ):
        load_sems = [
            ctx.enter_context(nc.semaphore(f"load{s}")) for s in range(B)
        ]
        store_sems = [
            ctx.enter_context(nc.semaphore(f"store{s}")) for s in range(B)
        ]

        def slot(t):
            base = (t % B) * _PACKB
            return (
                tiles[:, base : base + _XB].bitcast(f32),            # x
                tiles[:, base + _XB : base + _XB + _UB],             # uhat u8
                tiles[:, base + _XB + _UB : base + _PACKB].bitcast(f32),
                tiles[:, base : base + _PACKB],                      # raw slot
            )

        @block.sync
        def _(g):
            for t in range(_PT):
                s = t % B
                if t >= B:
                    # slot free once its previous store finished reading
                    g.wait_ge(store_sems[s], 16 * (t // B))
                g.dma_start(slot(t)[3], xu[bass.ts(t, 128), :]).then_inc(
                    load_sems[s], 16
                )

        @block.vector
        def _(v):
            for t in range(_PT):
                s = t % B
                v.wait_ge(load_sems[s], 16 * (t // B + 1))
                xs, us, ks, _raw = slot(t)
                v.scalar_tensor_tensor(
                    out=xs,
                    in0=us,
                    scalar=ks,
                    in1=xs,
                    op0=mybir.AluOpType.is_lt,
                    op1=mybir.AluOpType.mult,
                ).then_inc(dve_sem, 1)

        @block.scalar
        def _(s_eng):
            for t in range(_PT):
                s_eng.wait_ge(dve_sem, t + 1)
                s_eng.dma_start(out[bass.ts(t, 128), :], slot(t)[0]).then_inc(
                    store_sems[t % B], 16
                )
    return nc


def kernel(x, feature_importance):
    from concourse.bass_utils import run_bass_kernel_spmd

    if "nc" not in _state:
        _state["nc"] = _build_nc()
    nc = _state["nc"]

    st = _get_tables()
    buf = _get_buf()
    kp = _keep_prob(np.asarray(feature_importance))
    tauhat64 = np.floor(kp.astype(np.float64) * 256.0)     # exact
    tauhat32 = tauhat64.astype(np.float32)                 # <= 230, exact
    tauhat8 = tauhat64.astype(np.uint8)

    x_np = np.asarray(x, dtype=np.float32)
    f32v = buf.view(np.float32).reshape(_NCORES, _ROWS, _PACKB // 4)
    f32v[:, :, :_COLS] = x_np.reshape(_NCORES, _ROWS, _COLS)
    f32v[:, :, _PACKB // 4 - 1] = np.tile(tauhat32, _BPC)[None, :]

    in_maps = [{"xu": buf[k]} for k in range(_NCORES)]
    res = run_bass_kernel_spmd(
        nc, in_maps, core_ids=list(range(_NCORES)), trace=TRACE
    )
    _state["last"] = res
    out = np.concatenate([res.results[k]["out"] for k in range(_NCORES)], axis=0)
    outf = out.reshape(-1)

    # exact host patch of the undecided boundary bucket (uhat == tauhat_c)
    xf = x_np.reshape(-1)
    s8, sv, gi = st["sorted_u8"], st["sorted_u"], st["g_idx"]
    for c in range(_C):
        lo = np.searchsorted(s8[c], tauhat8[c], "left")
        hi = np.searchsorted(s8[c], tauhat8[c], "right")
        if lo == hi:
            continue
        g = gi[c, lo:hi]
        mask = (sv[c, lo:hi] < kp[c]).astype(np.float32)
        outf[g] = xf[g] * mask

    return out.reshape(_B, _C, _H, _W)
